# revision 7
# baseline (speedup 1.0000x reference)
"""CharBiLSTM Trainium2 kernel.

Strategy:
- Embedding lookup + input projection folded into G = emb @ W_ih.T + b (host),
  so per-step input contribution is a one-hot matmul (one-hot built on host).
- Words sorted by length into 256-wide bins (equal structure across the 8
  cores -> one SPMD program); each bin runs only max-len-in-bin steps and
  finished words' hidden states are emitted at their own final step.
- Feature-major LSTM state ([hid, words]) so the recurrence needs zero
  transposes; gates evacuated from PSUM by ScalarE with fused sigmoid/tanh.
- fp32r matmuls (full PE rate at N>=256, ~1e-4 relative error).
"""

import numpy as np

N_WORDS, MAX_LEN = 16384, 16
VOCAB, EMB, HID = 128, 64, 256
NCORES = 8
BIN_W = 256  # words per bin (free dim of all matmuls); PSUM-bank limited

_LAST_RESULT = {}  # test introspection: exec_time_ns etc.


def _build_schedule(lengths):
    """Sort words by length; build per-core column schedule.

    Returns:
      core_words: [NCORES][Q] word ids (-1 = dummy), identical len-structure
      col_lens:   [Q] length of each column (same for every core)
      bins:       list of (start_col, W, S) with W == BIN_W
    """
    lengths = np.asarray(lengths)
    per_core = [[] for _ in range(NCORES)]
    col_lens = []
    for L in range(1, MAX_LEN + 1):
        idx = np.where(lengths == L)[0]
        q = -(-len(idx) // NCORES)  # ceil
        pad = q * NCORES - len(idx)
        if pad:
            idx = np.concatenate([idx, np.full(pad, -1, np.int64)])
        for c in range(NCORES):
            per_core[c].extend(idx[c * q:(c + 1) * q].tolist())
        col_lens.extend([L] * q)
    Q = len(col_lens)
    nbins = -(-Q // BIN_W)
    tot = nbins * BIN_W
    for c in range(NCORES):
        per_core[c].extend([-1] * (tot - Q))
    # dummy cols get length 0 (never emitted, never one-hot)
    col_lens = col_lens + [0] * (tot - Q)
    col_lens = np.array(col_lens, np.int64)
    bins = []
    for b in range(nbins):
        sl = col_lens[b * BIN_W:(b + 1) * BIN_W]
        S = int(sl.max())
        bins.append((b * BIN_W, BIN_W, S))
    return [np.array(w, np.int64) for w in per_core], col_lens, bins


def _emit_ranges(col_lens, start, W, S):
    """For each step t (0-based), the [a,b) column range with len == t+1."""
    sl = col_lens[start:start + W]
    out = {}
    for t in range(S):
        cols = np.where(sl == t + 1)[0]
        if len(cols):
            a, b = int(cols[0]), int(cols[-1]) + 1
            assert b - a == len(cols), "columns of equal length must be contiguous"
            out[t] = (a, b)
    return out


def _build_bass(bins, emits, TOT, OUTCOLS):
    import concourse.bacc as bacc
    import concourse.tile as tile
    from concourse import mybir

    f32 = mybir.dt.float32
    f32r = mybir.dt.float32r
    Sig = mybir.ActivationFunctionType.Sigmoid
    Tanh = mybir.ActivationFunctionType.Tanh

    nc = bacc.Bacc(None, target_bir_lowering=False)
    d_oneh = nc.dram_tensor("oneh", [128, TOT], f32r, kind="ExternalInput")
    d_whh = nc.dram_tensor("whh", [128, 2 * 2 * 8 * 128], f32r, kind="ExternalInput")
    d_gt = nc.dram_tensor("gt", [128, 2 * 8 * 128], f32r, kind="ExternalInput")
    d_out = nc.dram_tensor("out", [128, OUTCOLS], f32, kind="ExternalOutput")

    whh_v = d_whh[:, :].rearrange("p (d k m c) -> p d k m c", d=2, k=2, m=8)
    gt_v = d_gt[:, :].rearrange("p (d m c) -> p d m c", d=2, m=8)

    with tile.TileContext(nc) as tc:
        with tc.tile_pool(name="wpool", bufs=1) as wpool, \
             tc.tile_pool(name="ohp", bufs=3) as ohp, \
             tc.tile_pool(name="psp", bufs=1, space="PSUM") as psp, \
             tc.tile_pool(name="actp", bufs=2) as actp, \
             tc.tile_pool(name="stp", bufs=2) as stp, \
             tc.tile_pool(name="tmpp", bufs=2) as tmpp:

            whh_sb = wpool.tile([128, 2, 2, 8, 128], f32r)
            nc.sync.dma_start(out=whh_sb, in_=whh_v)
            gt_sb = wpool.tile([128, 2, 8, 128], f32r)
            nc.sync.dma_start(out=gt_sb, in_=gt_v)

            oh_off = 0
            out_off = 0
            for bi, (start, W, S) in enumerate(bins):
                er = emits[bi]
                h = [None, None]
                c = [None, None]
                for t in range(S):
                    oh = ohp.tile([128, 2, W], f32r, tag="oh")
                    nc.sync.dma_start(
                        out=oh,
                        in_=d_oneh[:, oh_off:oh_off + 2 * W].rearrange(
                            "p (d w) -> p d w", d=2))
                    oh_off += 2 * W
                    for d in (0, 1):
                        ps_s = psp.tile([128, 6, W], f32, tag=f"pss{d}")
                        ps_g = psp.tile([128, 2, W], f32, tag=f"psg{d}")
                        for m in range(8):
                            o_ap = ps_s[:, m, :] if m < 6 else ps_g[:, m - 6, :]
                            nc.tensor.matmul(o_ap, gt_sb[:, d, m, :], oh[:, d, :],
                                             start=True, stop=(t == 0))
                            if t > 0:
                                nc.tensor.matmul(o_ap, whh_sb[:, d, 0, m, :],
                                                 h[d][:, 0, :],
                                                 start=False, stop=False)
                                nc.tensor.matmul(o_ap, whh_sb[:, d, 1, m, :],
                                                 h[d][:, 1, :],
                                                 start=False, stop=True)
                        sig = actp.tile([128, 6, W], f32, tag=f"sig{d}")
                        nc.scalar.activation(sig, ps_s, Sig)
                        tg = actp.tile([128, 2, W], f32, tag=f"tg{d}")
                        nc.scalar.activation(tg, ps_g, Tanh)
                        c_new = stp.tile([128, 2, W], f32, tag=f"c{d}")
                        if t == 0:
                            nc.vector.tensor_mul(c_new, sig[:, 0:2, :], tg)
                        else:
                            t1 = tmpp.tile([128, 2, W], f32, tag=f"t1{d}")
                            nc.vector.tensor_mul(t1, sig[:, 2:4, :], c[d])
                            t2 = tmpp.tile([128, 2, W], f32, tag=f"t2{d}")
                            nc.vector.tensor_mul(t2, sig[:, 0:2, :], tg)
                            nc.vector.tensor_add(c_new, t1, t2)
                        tc_t = tmpp.tile([128, 2, W], f32, tag=f"tc{d}")
                        nc.scalar.activation(tc_t, c_new, Tanh)
                        h_new = stp.tile([128, 2, W], f32r, tag=f"h{d}")
                        nc.vector.tensor_mul(h_new, sig[:, 4:6, :], tc_t)
                        h[d], c[d] = h_new, c_new
                        if t in er:
                            a, b = er[t]
                            dst = d_out[:, out_off + d * 2 * W:
                                        out_off + (d + 1) * 2 * W].rearrange(
                                "p (hh w) -> p hh w", hh=2)[:, :, a:b]
                            nc.sync.dma_start(out=dst,
                                              in_=h_new[:, :, a:b].bitcast(f32))
                out_off += 4 * W
    nc.compile()
    return nc


def _make_runner(nc, n_cores):
    """Build a reusable jitted SPMD executor for a compiled Bass module.

    Mirrors concourse.bass2jax.run_bass_via_pjrt's shard_map path, but
    keeps the jitted function so repeat calls (for timing) reuse the
    compiled NEFF instead of recompiling.
    """
    import jax
    from jax.sharding import Mesh, PartitionSpec
    from jax.experimental.shard_map import shard_map
    from concourse import bass2jax, mybir

    bass2jax.install_neuronx_cc_hook()
    assert nc.dbg_addr is None
    part_name = nc.partition_id_tensor.name if nc.partition_id_tensor else None

    in_names, out_names, out_avals, zero_outs = [], [], [], []
    for alloc in nc.m.functions[0].allocations:
        if not isinstance(alloc, mybir.MemoryLocationSet):
            continue
        name = alloc.memorylocations[0].name
        if alloc.kind == "ExternalInput":
            if name != part_name:
                in_names.append(name)
        elif alloc.kind == "ExternalOutput":
            np_dt = mybir.dt.np(alloc.dtype)
            shape = tuple(alloc.tensor_shape)
            out_avals.append(jax.core.ShapedArray(shape, np_dt))
            out_names.append(name)
            zero_outs.append(np.zeros(shape, np_dt))
    n_params = len(in_names)
    all_names = in_names + out_names
    if part_name is not None:
        all_names = all_names + [part_name]

    def _body(*args):
        operands = list(args)
        if part_name is not None:
            operands.append(bass2jax.partition_id_tensor())
        outs = bass2jax._bass_exec_p.bind(
            *operands,
            out_avals=tuple(out_avals),
            in_names=tuple(all_names),
            out_names=tuple(out_names),
            lowering_input_output_aliases=(),
            sim_require_finite=True,
            sim_require_nnan=True,
            nc=nc,
        )
        return tuple(outs)

    devices = jax.devices()[:n_cores]
    mesh = Mesh(np.asarray(devices), ("core",))
    nin = n_params + len(zero_outs)
    sharded = jax.jit(
        shard_map(_body, mesh=mesh,
                  in_specs=(PartitionSpec("core"),) * nin,
                  out_specs=(PartitionSpec("core"),) * len(out_names),
                  check_rep=False),
        keep_unused=True,
    )
    return sharded, in_names, out_names, out_avals, zero_outs


def _run_spmd(nc, in_maps, time_iters=0):
    """Execute once (returns per-core result dicts); optionally time."""
    import time as _time
    import jax

    n_cores = len(in_maps)
    sharded, in_names, out_names, out_avals, zero_outs = _make_runner(nc, n_cores)
    concat_in = [
        np.concatenate([np.asarray(in_maps[c][nm]) for c in range(n_cores)], axis=0)
        for nm in in_names
    ]
    concat_zeros = [
        np.zeros((n_cores * z.shape[0], *z.shape[1:]), z.dtype) for z in zero_outs
    ]
    dev_args = [jax.device_put(a) for a in concat_in + concat_zeros]
    out_arrs = sharded(*dev_args)
    jax.block_until_ready(out_arrs)

    exec_ns = None
    if time_iters:
        # warm
        jax.block_until_ready(sharded(*dev_args))
        t0 = _time.perf_counter()
        last = None
        for _ in range(time_iters):
            last = sharded(*dev_args)
        jax.block_until_ready(last)
        exec_ns = (_time.perf_counter() - t0) / time_iters * 1e9

    results = [
        {nm: np.asarray(out_arrs[i]).reshape(n_cores, *out_avals[i].shape)[c]
         for i, nm in enumerate(out_names)}
        for c in range(n_cores)
    ]
    return results, exec_ns


def kernel(char_ids, lengths, emb, W_ih_f, W_hh_f, b_ih_f, b_hh_f,
           W_ih_b, W_hh_b, b_ih_b, b_hh_b):
    char_ids = np.asarray(char_ids)
    lengths = np.asarray(lengths)

    # ---- host precompute: fold emb + input proj + biases into G [VOCAB, 4H]
    # permute gate order (i,f,g,o) -> (i,f,o,g) so ACT can evacuate
    # sigmoid-gates [i,f,o] with one instruction
    perm = np.concatenate([np.arange(0, 512),            # i, f
                           np.arange(768, 1024),         # o
                           np.arange(512, 768)])         # g
    outs = {}
    for d, (W_ih, W_hh, b_ih, b_hh) in enumerate(
            [(W_ih_f, W_hh_f, b_ih_f, b_hh_f),
             (W_ih_b, W_hh_b, b_ih_b, b_hh_b)]):
        G = (np.asarray(emb, np.float64) @ np.asarray(W_ih, np.float64).T
             + np.asarray(b_ih, np.float64) + np.asarray(b_hh, np.float64))
        outs[f"G{d}"] = np.ascontiguousarray(G[:, perm]).astype(np.float32)
        Wp = np.asarray(W_hh, np.float64)[perm, :].T  # [HID, 4H]
        outs[f"Wp{d}"] = Wp.astype(np.float32)

    # gt blob: [128, 2, 8, 128]
    gt = np.zeros((128, 2, 8, 128), np.float32)
    for d in range(2):
        for m in range(8):
            gt[:, d, m, :] = outs[f"G{d}"][:, m * 128:(m + 1) * 128]
    # whh blob: [128, 2, 2, 8, 128]
    whh = np.zeros((128, 2, 2, 8, 128), np.float32)
    for d in range(2):
        for k in range(2):
            for m in range(8):
                whh[:, d, k, m, :] = outs[f"Wp{d}"][
                    k * 128:(k + 1) * 128, m * 128:(m + 1) * 128]
    gt = gt.reshape(128, -1)
    whh = whh.reshape(128, -1)

    # ---- schedule
    core_words, col_lens, bins = _build_schedule(lengths)
    emits = [_emit_ranges(col_lens, s, W, S) for (s, W, S) in bins]
    TOT = sum(2 * W * S for (_, W, S) in bins)
    OUTCOLS = sum(4 * W for (_, W, _) in bins)

    # ---- one-hot blobs per core
    in_maps = []
    for cidx in range(NCORES):
        words = core_words[cidx]
        oh = np.zeros((128, TOT), np.float32)
        off = 0
        for (start, W, S) in bins:
            w_ids = words[start:start + W]
            lens = col_lens[start:start + W]
            cols = np.arange(W)
            real = w_ids >= 0
            for t in range(S):
                valid = real & (t < lens)
                if valid.any():
                    wv = w_ids[valid]
                    # fwd: char at position t
                    rows_f = char_ids[wv, t]
                    oh[rows_f, off + cols[valid]] = 1.0
                    # bwd: char at position len-1-t
                    rows_b = char_ids[wv, lens[valid] - 1 - t]
                    oh[rows_b, off + W + cols[valid]] = 1.0
                off += 2 * W
        in_maps.append({"oneh": oh, "whh": whh, "gt": gt})

    # ---- build + run
    import os
    nc = _build_bass(bins, emits, TOT, OUTCOLS)
    iters = int(os.environ.get("KERNEL_TIME_ITERS", "0"))
    results, exec_ns = _run_spmd(nc, in_maps, time_iters=iters)
    _LAST_RESULT.clear()
    _LAST_RESULT["exec_time_ns"] = exec_ns

    # ---- assemble output
    final = np.zeros((N_WORDS, 2 * HID), np.float32)
    for cidx in range(NCORES):
        out = results[cidx]["out"]
        words = core_words[cidx]
        ob = 0
        for (start, W, S) in bins:
            w_ids = words[start:start + W]
            real = w_ids >= 0
            for d in range(2):
                block = out[:, ob + d * 2 * W: ob + (d + 1) * 2 * W]
                hv = block.reshape(128, 2, W).transpose(2, 1, 0).reshape(W, 256)
                final[w_ids[real], d * HID:(d + 1) * HID] = hv[real]
            ob += 4 * W
    return final


# revision 8
# speedup vs baseline: 1.3051x; 1.3051x over previous
"""CharBiLSTM Trainium2 kernel.

Strategy:
- Embedding lookup + input projection folded into G = emb @ W_ih.T + b (host),
  so per-step input contribution is a one-hot matmul (one-hot built on host).
- Words sorted by length into 256-wide bins (equal structure across the 8
  cores -> one SPMD program); each bin runs only max-len-in-bin steps and
  finished words' hidden states are emitted at their own final step.
- Feature-major LSTM state ([hid, words]) so the recurrence needs zero
  transposes; gates evacuated from PSUM by ScalarE with fused sigmoid/tanh.
- fp32r matmuls (full PE rate at N>=256, ~1e-4 relative error).
"""

import numpy as np

N_WORDS, MAX_LEN = 16384, 16
VOCAB, EMB, HID = 128, 64, 256
NCORES = 8
BIN_W = 256  # words per bin (free dim of all matmuls); PSUM-bank limited

_LAST_RESULT = {}  # test introspection: exec_time_ns etc.


def _build_schedule(lengths):
    """Sort words by length; build per-core column schedule.

    Returns:
      core_words: [NCORES][Q] word ids (-1 = dummy), identical len-structure
      col_lens:   [Q] length of each column (same for every core)
      bins:       list of (start_col, W, S) with W == BIN_W
    """
    lengths = np.asarray(lengths)
    per_core = [[] for _ in range(NCORES)]
    col_lens = []
    # descending length order: the ragged tail bin (dummy-padded) then has
    # S=1 instead of S=16
    for L in range(MAX_LEN, 0, -1):
        idx = np.where(lengths == L)[0]
        q = -(-len(idx) // NCORES)  # ceil
        pad = q * NCORES - len(idx)
        if pad:
            idx = np.concatenate([idx, np.full(pad, -1, np.int64)])
        for c in range(NCORES):
            per_core[c].extend(idx[c * q:(c + 1) * q].tolist())
        col_lens.extend([L] * q)
    Q = len(col_lens)
    nbins = -(-Q // BIN_W)
    tot = nbins * BIN_W
    for c in range(NCORES):
        per_core[c].extend([-1] * (tot - Q))
    # dummy cols get length 0 (never emitted, never one-hot)
    col_lens = col_lens + [0] * (tot - Q)
    col_lens = np.array(col_lens, np.int64)
    bins = []
    for b in range(nbins):
        sl = col_lens[b * BIN_W:(b + 1) * BIN_W]
        S = int(sl.max())
        bins.append((b * BIN_W, BIN_W, S))
    return [np.array(w, np.int64) for w in per_core], col_lens, bins


def _emit_ranges(col_lens, start, W, S):
    """For each step t (0-based), the [a,b) column range with len == t+1."""
    sl = col_lens[start:start + W]
    out = {}
    for t in range(S):
        cols = np.where(sl == t + 1)[0]
        if len(cols):
            a, b = int(cols[0]), int(cols[-1]) + 1
            assert b - a == len(cols), "columns of equal length must be contiguous"
            out[t] = (a, b)
    return out


def _build_bass(bins, emits, TOT, OUTCOLS):
    import concourse.bacc as bacc
    import concourse.tile as tile
    from concourse import mybir

    f32 = mybir.dt.float32
    f32r = mybir.dt.float32r
    Sig = mybir.ActivationFunctionType.Sigmoid
    Tanh = mybir.ActivationFunctionType.Tanh

    nc = bacc.Bacc(None, target_bir_lowering=False)
    d_oneh = nc.dram_tensor("oneh", [128, TOT], f32r, kind="ExternalInput")
    d_whh = nc.dram_tensor("whh", [128, 2 * 2 * 8 * 128], f32r, kind="ExternalInput")
    d_gt = nc.dram_tensor("gt", [128, 2 * 8 * 128], f32r, kind="ExternalInput")
    d_out = nc.dram_tensor("out", [128, OUTCOLS], f32, kind="ExternalOutput")

    whh_v = d_whh[:, :].rearrange("p (d k m c) -> p d k m c", d=2, k=2, m=8)
    gt_v = d_gt[:, :].rearrange("p (d m c) -> p d m c", d=2, m=8)

    with tile.TileContext(nc) as tc:
        with tc.tile_pool(name="wpool", bufs=1) as wpool, \
             tc.tile_pool(name="ohp", bufs=3) as ohp, \
             tc.tile_pool(name="psp", bufs=1, space="PSUM") as psp, \
             tc.tile_pool(name="actp", bufs=2) as actp, \
             tc.tile_pool(name="stp", bufs=2) as stp, \
             tc.tile_pool(name="tmpp", bufs=2) as tmpp:

            whh_sb = wpool.tile([128, 2, 2, 8, 128], f32r)
            nc.sync.dma_start(out=whh_sb, in_=whh_v)
            gt_sb = wpool.tile([128, 2, 8, 128], f32r)
            nc.sync.dma_start(out=gt_sb, in_=gt_v)

            oh_off = 0
            out_off = 0
            for bi, (start, W, S) in enumerate(bins):
                er = emits[bi]
                h = [None, None]
                c = [None, None]
                for t in range(S):
                    oh = ohp.tile([128, 2, W], f32r, tag="oh")
                    nc.sync.dma_start(
                        out=oh,
                        in_=d_oneh[:, oh_off:oh_off + 2 * W].rearrange(
                            "p (d w) -> p d w", d=2))
                    oh_off += 2 * W
                    for d in (0, 1):
                        ps_s = psp.tile([128, 6, W], f32, tag=f"pss{d}")
                        ps_g = psp.tile([128, 2, W], f32, tag=f"psg{d}")
                        for m in range(8):
                            o_ap = ps_s[:, m, :] if m < 6 else ps_g[:, m - 6, :]
                            nc.tensor.matmul(o_ap, gt_sb[:, d, m, :], oh[:, d, :],
                                             start=True, stop=(t == 0))
                            if t > 0:
                                nc.tensor.matmul(o_ap, whh_sb[:, d, 0, m, :],
                                                 h[d][:, 0, :],
                                                 start=False, stop=False)
                                nc.tensor.matmul(o_ap, whh_sb[:, d, 1, m, :],
                                                 h[d][:, 1, :],
                                                 start=False, stop=True)
                        sig = actp.tile([128, 6, W], f32, tag=f"sig{d}")
                        nc.scalar.activation(sig, ps_s, Sig)
                        tg = actp.tile([128, 2, W], f32, tag=f"tg{d}")
                        nc.scalar.activation(tg, ps_g, Tanh)
                        c_new = stp.tile([128, 2, W], f32, tag=f"c{d}")
                        if t == 0:
                            nc.vector.tensor_mul(c_new, sig[:, 0:2, :], tg)
                        else:
                            t1 = tmpp.tile([128, 2, W], f32, tag=f"t1{d}")
                            nc.vector.tensor_mul(t1, sig[:, 2:4, :], c[d])
                            t2 = tmpp.tile([128, 2, W], f32, tag=f"t2{d}")
                            nc.vector.tensor_mul(t2, sig[:, 0:2, :], tg)
                            nc.vector.tensor_add(c_new, t1, t2)
                        tc_t = tmpp.tile([128, 2, W], f32, tag=f"tc{d}")
                        nc.scalar.activation(tc_t, c_new, Tanh)
                        h_new = stp.tile([128, 2, W], f32r, tag=f"h{d}")
                        nc.vector.tensor_mul(h_new, sig[:, 4:6, :], tc_t)
                        h[d], c[d] = h_new, c_new
                        if t in er:
                            a, b = er[t]
                            dst = d_out[:, out_off + d * 2 * W:
                                        out_off + (d + 1) * 2 * W].rearrange(
                                "p (hh w) -> p hh w", hh=2)[:, :, a:b]
                            nc.sync.dma_start(out=dst,
                                              in_=h_new[:, :, a:b].bitcast(f32))
                out_off += 4 * W
    nc.compile()
    return nc


def _make_runner(nc, n_cores):
    """Build a reusable jitted SPMD executor for a compiled Bass module.

    Mirrors concourse.bass2jax.run_bass_via_pjrt's shard_map path, but
    keeps the jitted function so repeat calls (for timing) reuse the
    compiled NEFF instead of recompiling.
    """
    import jax
    from jax.sharding import Mesh, PartitionSpec
    from jax.experimental.shard_map import shard_map
    from concourse import bass2jax, mybir

    bass2jax.install_neuronx_cc_hook()
    assert nc.dbg_addr is None
    part_name = nc.partition_id_tensor.name if nc.partition_id_tensor else None

    in_names, out_names, out_avals, zero_outs = [], [], [], []
    for alloc in nc.m.functions[0].allocations:
        if not isinstance(alloc, mybir.MemoryLocationSet):
            continue
        name = alloc.memorylocations[0].name
        if alloc.kind == "ExternalInput":
            if name != part_name:
                in_names.append(name)
        elif alloc.kind == "ExternalOutput":
            np_dt = mybir.dt.np(alloc.dtype)
            shape = tuple(alloc.tensor_shape)
            out_avals.append(jax.core.ShapedArray(shape, np_dt))
            out_names.append(name)
            zero_outs.append(np.zeros(shape, np_dt))
    n_params = len(in_names)
    all_names = in_names + out_names
    if part_name is not None:
        all_names = all_names + [part_name]

    def _body(*args):
        operands = list(args)
        if part_name is not None:
            operands.append(bass2jax.partition_id_tensor())
        outs = bass2jax._bass_exec_p.bind(
            *operands,
            out_avals=tuple(out_avals),
            in_names=tuple(all_names),
            out_names=tuple(out_names),
            lowering_input_output_aliases=(),
            sim_require_finite=True,
            sim_require_nnan=True,
            nc=nc,
        )
        return tuple(outs)

    devices = jax.devices()[:n_cores]
    mesh = Mesh(np.asarray(devices), ("core",))
    nin = n_params + len(zero_outs)
    sharded = jax.jit(
        shard_map(_body, mesh=mesh,
                  in_specs=(PartitionSpec("core"),) * nin,
                  out_specs=(PartitionSpec("core"),) * len(out_names),
                  check_rep=False),
        keep_unused=True,
    )
    return sharded, in_names, out_names, out_avals, zero_outs


def _run_spmd(nc, in_maps, time_iters=0):
    """Execute once (returns per-core result dicts); optionally time."""
    import time as _time
    import jax

    n_cores = len(in_maps)
    sharded, in_names, out_names, out_avals, zero_outs = _make_runner(nc, n_cores)
    concat_in = [
        np.concatenate([np.asarray(in_maps[c][nm]) for c in range(n_cores)], axis=0)
        for nm in in_names
    ]
    concat_zeros = [
        np.zeros((n_cores * z.shape[0], *z.shape[1:]), z.dtype) for z in zero_outs
    ]
    dev_args = [jax.device_put(a) for a in concat_in + concat_zeros]
    out_arrs = sharded(*dev_args)
    jax.block_until_ready(out_arrs)

    exec_ns = None
    if time_iters:
        # warm
        jax.block_until_ready(sharded(*dev_args))
        t0 = _time.perf_counter()
        last = None
        for _ in range(time_iters):
            last = sharded(*dev_args)
        jax.block_until_ready(last)
        exec_ns = (_time.perf_counter() - t0) / time_iters * 1e9

    results = [
        {nm: np.asarray(out_arrs[i]).reshape(n_cores, *out_avals[i].shape)[c]
         for i, nm in enumerate(out_names)}
        for c in range(n_cores)
    ]
    return results, exec_ns


def kernel(char_ids, lengths, emb, W_ih_f, W_hh_f, b_ih_f, b_hh_f,
           W_ih_b, W_hh_b, b_ih_b, b_hh_b):
    char_ids = np.asarray(char_ids)
    lengths = np.asarray(lengths)

    # ---- host precompute: fold emb + input proj + biases into G [VOCAB, 4H]
    # permute gate order (i,f,g,o) -> (i,f,o,g) so ACT can evacuate
    # sigmoid-gates [i,f,o] with one instruction
    perm = np.concatenate([np.arange(0, 512),            # i, f
                           np.arange(768, 1024),         # o
                           np.arange(512, 768)])         # g
    outs = {}
    for d, (W_ih, W_hh, b_ih, b_hh) in enumerate(
            [(W_ih_f, W_hh_f, b_ih_f, b_hh_f),
             (W_ih_b, W_hh_b, b_ih_b, b_hh_b)]):
        G = (np.asarray(emb, np.float64) @ np.asarray(W_ih, np.float64).T
             + np.asarray(b_ih, np.float64) + np.asarray(b_hh, np.float64))
        outs[f"G{d}"] = np.ascontiguousarray(G[:, perm]).astype(np.float32)
        Wp = np.asarray(W_hh, np.float64)[perm, :].T  # [HID, 4H]
        outs[f"Wp{d}"] = Wp.astype(np.float32)

    # gt blob: [128, 2, 8, 128]
    gt = np.zeros((128, 2, 8, 128), np.float32)
    for d in range(2):
        for m in range(8):
            gt[:, d, m, :] = outs[f"G{d}"][:, m * 128:(m + 1) * 128]
    # whh blob: [128, 2, 2, 8, 128]
    whh = np.zeros((128, 2, 2, 8, 128), np.float32)
    for d in range(2):
        for k in range(2):
            for m in range(8):
                whh[:, d, k, m, :] = outs[f"Wp{d}"][
                    k * 128:(k + 1) * 128, m * 128:(m + 1) * 128]
    gt = gt.reshape(128, -1)
    whh = whh.reshape(128, -1)

    # ---- schedule
    core_words, col_lens, bins = _build_schedule(lengths)
    emits = [_emit_ranges(col_lens, s, W, S) for (s, W, S) in bins]
    TOT = sum(2 * W * S for (_, W, S) in bins)
    OUTCOLS = sum(4 * W for (_, W, _) in bins)

    # ---- one-hot blobs per core
    in_maps = []
    for cidx in range(NCORES):
        words = core_words[cidx]
        oh = np.zeros((128, TOT), np.float32)
        off = 0
        for (start, W, S) in bins:
            w_ids = words[start:start + W]
            lens = col_lens[start:start + W]
            cols = np.arange(W)
            real = w_ids >= 0
            for t in range(S):
                valid = real & (t < lens)
                if valid.any():
                    wv = w_ids[valid]
                    # fwd: char at position t
                    rows_f = char_ids[wv, t]
                    oh[rows_f, off + cols[valid]] = 1.0
                    # bwd: char at position len-1-t
                    rows_b = char_ids[wv, lens[valid] - 1 - t]
                    oh[rows_b, off + W + cols[valid]] = 1.0
                off += 2 * W
        in_maps.append({"oneh": oh, "whh": whh, "gt": gt})

    # ---- build + run
    import os
    nc = _build_bass(bins, emits, TOT, OUTCOLS)
    iters = int(os.environ.get("KERNEL_TIME_ITERS", "0"))
    results, exec_ns = _run_spmd(nc, in_maps, time_iters=iters)
    _LAST_RESULT.clear()
    _LAST_RESULT["exec_time_ns"] = exec_ns

    # ---- assemble output
    final = np.zeros((N_WORDS, 2 * HID), np.float32)
    for cidx in range(NCORES):
        out = results[cidx]["out"]
        words = core_words[cidx]
        ob = 0
        for (start, W, S) in bins:
            w_ids = words[start:start + W]
            real = w_ids >= 0
            for d in range(2):
                block = out[:, ob + d * 2 * W: ob + (d + 1) * 2 * W]
                hv = block.reshape(128, 2, W).transpose(2, 1, 0).reshape(W, 256)
                final[w_ids[real], d * HID:(d + 1) * HID] = hv[real]
            ob += 4 * W
    return final
